# revision 1
# baseline (speedup 1.0000x reference)
"""Trainium2 Bass kernel for nn_CriterionLP (hardest-pos/hardest-neg LP loss).

Math (reference):
    sim  = feats @ feats_s.reshape(B*TOPK, C).T          # [B, B*TOPK]
    blk  = exp(sim/T).reshape(B, P_IDS, K_INST*TOPK)
    pos  = min over own identity block                    # exp is monotone =>
    nmax = max over each identity block                   #   reduce raw sim, exp later
    loss = mean(-log(pos / (pos + sum_{j!=pid} nmax_j + eps) + eps))

Device strategy (8 NeuronCores, SPMD — one program, per-core data):
  * Shard the support dim: core c owns support columns [4096c, 4096(c+1))
    (= identity blocks [32c, 32c+32)); each core sees all B anchors.
  * Anchors are rotated by 512c per core so each core's "own block" diagonal
    band sits at local anchor tiles 0..3 / local blocks [8a, 8a+8) — the
    program is identical across cores.
  * Per core: [C=128 x 4096] @ [C=128 x 4096] fp32r matmuls into PSUM,
    DVE segmented tensor_reduce (max per 128-col identity block; min on the
    diagonal band only), DMA [4096, 32] block-max + [512, 8] band-min out.
  * Host: gather, exp at the [B, 256] level, assemble the scalar loss.
"""

import numpy as np

B = 4096
C = 128
TOPK = 8
K_INST = 16
P_IDS = B // K_INST            # 256 identity blocks
BLK = K_INST * TOPK            # 128 support cols per identity block
TEMP = 0.05
EPS = 1e-6

N_CORES = 8
S_LOC = B * TOPK // N_CORES    # 4096 support cols per core
NBLK_LOC = S_LOC // BLK        # 32 identity blocks per core
A_ROT = B // N_CORES           # 512: per-core anchor rotation
ATILE = 128                    # anchors per tile (partition dim)
N_ATILES = B // ATILE          # 32
BPT = ATILE // K_INST          # 8 own-band blocks per anchor tile

_CACHE = {}

# Greedy DVE/ACT load balance (measured ns per half-tile).
DVE_DIRECT = 2280.0   # tensor_reduce [128, 16x128] from PSUM
DVE_MIN = 1190.0      # extra band min reduce from PSUM
DVE_TREE = 1670.0     # 4 fp16 2x folds to width 8 (host finishes 8->1)
ACT_CAST = 2160.0     # PSUM f32 -> SBUF fp16 copy (measured)


def _schedule():
    """Static DVE/ACT assignment; must match between build and host gather."""
    direct_map = {}
    dve_load, act_load = 0.0, 0.0
    for a in range(N_ATILES):
        for h in range(2):
            diag = a < 4 and h == a // 2
            direct = diag or (dve_load + DVE_DIRECT <= act_load + ACT_CAST)
            if direct:
                dve_load += DVE_DIRECT + (DVE_MIN if diag else 0.0)
            else:
                act_load += ACT_CAST
                dve_load += DVE_TREE
            direct_map[(a, h)] = direct
    return direct_map


def _build_program():
    import concourse.tile as tile
    from concourse import bacc, mybir
    from concourse.bass import ds, ts

    nc = bacc.Bacc(
        "TRN2", target_bir_lowering=False, debug=False, num_devices=N_CORES
    )
    f32 = mybir.dt.float32
    f16 = mybir.dt.float16
    X = mybir.AxisListType.X

    ft = nc.dram_tensor("ft", [C, B], f16, kind="ExternalInput").ap()
    st = nc.dram_tensor("st", [C, S_LOC], f16, kind="ExternalInput").ap()
    bmax = nc.dram_tensor("bmax", [B, NBLK_LOC], f32, kind="ExternalOutput").ap()
    bmin = nc.dram_tensor("bmin", [A_ROT, BPT], f32, kind="ExternalOutput").ap()
    bm8 = nc.dram_tensor("bm8", [B, 2, 16, 8], f16, kind="ExternalOutput").ap()

    direct_map = _schedule()

    with tile.TileContext(nc) as tc:
        with (
            tc.tile_pool(name="inp", bufs=1) as inp,
            tc.tile_pool(name="res", bufs=4) as resp,
            tc.tile_pool(name="minres", bufs=2) as minp,
            tc.tile_pool(name="cast", bufs=7) as castp,
            tc.tile_pool(name="tree", bufs=6) as treep,
            tc.tile_pool(name="psum", bufs=2, space="PSUM") as pp,
        ):
            ft_r = inp.tile([C, B], f16)
            st_r = inp.tile([C, S_LOC], f16)
            # First anchor tile needs ft[:, 0:128] and st[:, 0:2048]; emit
            # those pieces first so the pipeline starts as soon as possible.
            nc.sync.dma_start(ft_r[:, ts(0, 512)], ft[:, ts(0, 512)])
            for q in range(4):
                nc.sync.dma_start(
                    st_r[:, ts(q, S_LOC // 4)], st[:, ts(q, S_LOC // 4)]
                )
            for q in range(1, 8):
                nc.sync.dma_start(ft_r[:, ts(q, 512)], ft[:, ts(q, 512)])

            # PE HAM warm-up: ~7us of back-to-back dummy matmuls during the
            # input-DMA window flips the clock gate to 8/8 (2.4 GHz) before
            # the real work starts; steady-state gaps are short enough to
            # stay warm after that. Reads an uninitialized scratch tile (no
            # deps -> scheduled first), writes the first PSUM slot.
            warm = inp.tile([C, 512], f16)
            nc.scalar.memzero(warm[:])
            ps_w = pp.tile([ATILE, 4 * 512], f32, tag="ps")
            for i in range(7):
                nc.tensor.matmul(
                    ps_w[:, ts(i % 4, 512)],
                    warm[:, 0:ATILE],
                    warm[:],
                    start=True,
                    stop=True,
                )

            for a in range(N_ATILES):
                for h in range(2):  # two PSUM halves of 2048 support cols
                    ps = pp.tile([ATILE, 4 * 512], f32, tag="ps")
                    for j in range(4):
                        nc.tensor.matmul(
                            ps[:, ts(j, 512)],
                            ft_r[:, ts(a, ATILE)],
                            st_r[:, ds(2048 * h + 512 * j, 512)],
                            start=True,
                            stop=True,
                        )
                    diag = a < 4 and h == a // 2
                    if direct_map[(a, h)]:
                        res = resp.tile([ATILE, 16], f32)
                        nc.vector.tensor_reduce(
                            res[:],
                            ps[:].rearrange("p (b x) -> p b x", x=BLK),
                            axis=X,
                            op=mybir.AluOpType.max,
                        )
                        nc.sync.dma_start(
                            bmax[ts(a, ATILE), ds(16 * h, 16)], res[:]
                        )
                        if diag:
                            # own-block band: local blocks [8a, 8a+8)
                            mres = minp.tile([ATILE, BPT], f32)
                            nc.vector.tensor_reduce(
                                mres[:],
                                ps[:, ds((a % 2) * 1024, 1024)].rearrange(
                                    "p (b x) -> p b x", x=BLK
                                ),
                                axis=X,
                                op=mybir.AluOpType.min,
                            )
                            nc.sync.dma_start(bmin[ts(a, ATILE), :], mres[:])
                    else:
                        s = castp.tile([ATILE, 16, BLK], f16)
                        nc.scalar.copy(s[:], ps[:].rearrange("p (b x) -> p b x", x=BLK))
                        # fp16 2x pairwise-max folds down to width 8;
                        # the host finishes the last 8->1 reduction.
                        cur = s
                        for w in (64, 32, 16, 8):
                            nxt = treep.tile([ATILE, 16, w], f16, tag=f"tree{w}")
                            nc.vector.tensor_tensor(
                                nxt[:],
                                cur[:, :, 0:w],
                                cur[:, :, w : 2 * w],
                                op=mybir.AluOpType.max,
                            )
                            cur = nxt
                        nc.sync.dma_start(bm8[ts(a, ATILE), h, :, :], cur[:])

    nc.compile()
    return nc


def _get_program():
    if "nc" not in _CACHE:
        _CACHE["nc"] = _build_program()
    return _CACHE["nc"]


def _make_in_maps(feats, feats_s):
    fs = feats_s.reshape(B * TOPK, C)
    in_maps = []
    for c in range(N_CORES):
        ftc = np.ascontiguousarray(np.roll(feats, -A_ROT * c, axis=0).T).astype(
            np.float16
        )
        stc = np.ascontiguousarray(fs[S_LOC * c : S_LOC * (c + 1)].T).astype(
            np.float16
        )
        in_maps.append({"ft": ftc, "st": stc})
    return in_maps


def run_device(feats, feats_s, trace=False, tmpdir=None):
    """Run the SPMD program; return (blk_smax [B, P_IDS], pos_sim [B], raw)."""
    from concourse.bass_utils import run_bass_kernel_spmd

    nc = _get_program()
    in_maps = _make_in_maps(feats, feats_s)
    kw = {}
    if trace:
        kw = dict(trace=True, tmpdir=tmpdir)
    r = run_bass_kernel_spmd(nc, in_maps, list(range(N_CORES)), **kw)

    direct_map = _schedule()
    blk_smax = np.empty((B, P_IDS), np.float64)
    pos_sim = np.empty((B,), np.float64)
    i = np.arange(A_ROT)
    for c in range(N_CORES):
        bm = np.array(r.results[c]["bmax"])    # [B, 32]; valid on direct halves
        bm8 = np.asarray(r.results[c]["bm8"])  # [B, 2, 16, 8] fp16 tree tops
        bm8 = bm8.astype(np.float32).max(axis=3)  # [B, 2, 16]
        for a in range(N_ATILES):
            for h in range(2):
                if not direct_map[(a, h)]:
                    bm[128 * a : 128 * (a + 1), 16 * h : 16 * (h + 1)] = bm8[
                        128 * a : 128 * (a + 1), h
                    ]
        blk_smax[:, NBLK_LOC * c : NBLK_LOC * (c + 1)] = np.roll(
            bm, A_ROT * c, axis=0
        )
        mn = np.asarray(r.results[c]["bmin"])  # [512, 8] band mins
        pos_sim[A_ROT * c + i] = mn[i, (i // K_INST) % BPT]
    return blk_smax, pos_sim, r


def _loss_from_reductions(blk_smax, pos_sim, labels):
    e = np.exp(blk_smax / TEMP)             # [B, P_IDS] block max of exp
    own = e[np.arange(B), labels]
    neg = e.sum(axis=1) - own
    pos = np.exp(pos_sim / TEMP)
    loss = -np.log(pos / (pos + neg + EPS) + EPS)
    return np.float32(loss.mean())


def _numpy_fallback(feats, feats_s, labels):
    # Exact mirror of the reference, host-only. Safety net for label
    # patterns other than arange(B)//K_INST (never produced by setup_inputs).
    fs = feats_s.reshape(B * TOPK, C)
    out = np.empty((B,), np.float64)
    sim = feats.astype(np.float64) @ fs.astype(np.float64).T
    e = np.exp(sim / TEMP).reshape(B, P_IDS, BLK)
    pos = e[np.arange(B), labels].min(axis=1)
    bm = e.max(axis=2)
    neg = bm.sum(axis=1) - bm[np.arange(B), labels]
    out = -np.log(pos / (pos + neg + EPS) + EPS)
    return np.float32(out.mean())


def kernel(**inputs):
    feats = np.ascontiguousarray(np.asarray(inputs["feats"], dtype=np.float32))
    feats_s = np.ascontiguousarray(np.asarray(inputs["feats_s"], dtype=np.float32))
    labels = np.asarray(inputs["labels"]).astype(np.int64)

    blk_smax, pos_sim, _ = run_device(feats, feats_s)

    if not np.array_equal(labels, np.arange(B, dtype=np.int64) // K_INST):
        return _numpy_fallback(feats, feats_s, labels)
    return _loss_from_reductions(blk_smax, pos_sim, labels)



# revision 3
# speedup vs baseline: 1.1158x; 1.1158x over previous
"""Trainium2 Bass kernel for nn_CriterionLP (hardest-pos/hardest-neg LP loss).

Math (reference):
    sim  = feats @ feats_s.reshape(B*TOPK, C).T          # [B, B*TOPK]
    blk  = exp(sim/T).reshape(B, P_IDS, K_INST*TOPK)
    pos  = min over own identity block                    # exp is monotone =>
    nmax = max over each identity block                   #   reduce raw sim, exp later
    loss = mean(-log(pos / (pos + sum_{j!=pid} nmax_j + eps) + eps))

Device strategy (8 NeuronCores, SPMD — one program, per-core data):
  * Shard the support dim: core c owns support columns [4096c, 4096(c+1));
    anchors are rotated by 512c per core so the program is identical per core.
  * Per core: [C=128 x 4096] @ [C=128 x 4096] fp16 matmuls into PSUM.
    The [128, 2048] PSUM halves are drained by two balanced engine paths:
      DIRECT: DVE segmented tensor_reduce(max) straight from PSUM (2.28us)
      CAST64: ACT copies PSUM f32 -> SBUF fp16 (2.0us), one DVE fp16
              tensor_tensor fold 128->64 (0.7us), ship [128,16,64] fp16;
              the host finishes the 64->1 max (fp16 rounding commutes with
              max, so only the final block max is rounded once).
      RAW:    the 4 diagonal tiles (which also need the own-block band min)
              are ACT-cast and shipped raw [128,16,128]; the host computes
              both the block maxes and the band min. Zero DVE work.
    A greedy schedule balances DVE vs ACT finishing times.
  * Host: gather, exp at the [B, 256] level, assemble the scalar loss.
"""

import numpy as np

B = 4096
C = 128
TOPK = 8
K_INST = 16
P_IDS = B // K_INST            # 256 identity blocks
BLK = K_INST * TOPK            # 128 support cols per identity block
TEMP = 0.05
EPS = 1e-6

N_CORES = 8
S_LOC = B * TOPK // N_CORES    # 4096 support cols per core
NBLK_LOC = S_LOC // BLK        # 32 identity blocks per core
A_ROT = B // N_CORES           # 512: per-core anchor rotation
ATILE = 128                    # anchors per tile (partition dim)
N_ATILES = B // ATILE          # 32
BPT = ATILE // K_INST          # 8 own-band blocks per anchor tile

_CACHE = {}

# Measured per-half-tile engine costs (ns), from the baseline NTFF trace.
DVE_DIRECT = 2280.0   # tensor_reduce [128, 16x128] f32 from PSUM
ACT_CAST = 1977.0     # ACT copy PSUM f32 -> SBUF fp16 [128, 2048]
DVE_FOLD1 = 701.0     # fp16 TT max fold 128 -> 64 from SBUF

DIRECT, CAST64, RAW = 0, 1, 2


def _schedule():
    """Greedy DVE/ACT balance; must match between build and host gather."""
    tmap = {}
    dve, act = 0.0, 0.0
    for a in range(N_ATILES):
        for h in range(2):
            diag = a < 4 and h == a // 2
            if diag:
                tmap[(a, h)] = RAW
                act += ACT_CAST
                continue
            m_dir = max(dve + DVE_DIRECT, act)
            m_cast = max(dve + DVE_FOLD1, act + ACT_CAST)
            if m_dir <= m_cast:
                tmap[(a, h)] = DIRECT
                dve += DVE_DIRECT
            else:
                tmap[(a, h)] = CAST64
                dve += DVE_FOLD1
                act += ACT_CAST
    return tmap


def _build_program():
    import concourse.tile as tile
    from concourse import bacc, mybir
    from concourse.bass import ds, ts

    nc = bacc.Bacc(
        "TRN2", target_bir_lowering=False, debug=False, num_devices=N_CORES
    )
    f32 = mybir.dt.float32
    f16 = mybir.dt.float16
    X = mybir.AxisListType.X

    ft = nc.dram_tensor("ft", [C, B], f16, kind="ExternalInput").ap()
    st = nc.dram_tensor("st", [C, S_LOC], f16, kind="ExternalInput").ap()
    bmax = nc.dram_tensor("bmax", [B, NBLK_LOC], f32, kind="ExternalOutput").ap()
    bm64 = nc.dram_tensor("bm64", [B, 2, 16, 64], f16, kind="ExternalOutput").ap()
    braw = nc.dram_tensor("braw", [4 * ATILE, 16, BLK], f16, kind="ExternalOutput").ap()

    tmap = _schedule()

    with tile.TileContext(nc) as tc:
        with (
            tc.tile_pool(name="inp", bufs=1) as inp,
            tc.tile_pool(name="res", bufs=4) as resp,
            tc.tile_pool(name="cast", bufs=6) as castp,
            tc.tile_pool(name="tree", bufs=4) as treep,
            tc.tile_pool(name="psum", bufs=2, space="PSUM") as pp,
        ):
            ft_r = inp.tile([C, B], f16)
            st_r = inp.tile([C, S_LOC], f16)
            # First anchor tile needs ft[:, 0:128] and st[:, 0:2048]; emit
            # those pieces first so the pipeline starts as soon as possible.
            nc.sync.dma_start(ft_r[:, ts(0, 512)], ft[:, ts(0, 512)])
            for q in range(4):
                nc.sync.dma_start(
                    st_r[:, ts(q, S_LOC // 4)], st[:, ts(q, S_LOC // 4)]
                )
            for q in range(1, 8):
                nc.sync.dma_start(ft_r[:, ts(q, 512)], ft[:, ts(q, 512)])

            # PE HAM warm-up: back-to-back dummy matmuls during the input-DMA
            # window flip the clock gate to 8/8 before the real work starts.
            warm = inp.tile([C, 512], f16)
            nc.scalar.memzero(warm[:])
            ps_w = pp.tile([ATILE, 4 * 512], f32, tag="ps")
            for i in range(7):
                nc.tensor.matmul(
                    ps_w[:, ts(i % 4, 512)],
                    warm[:, 0:ATILE],
                    warm[:],
                    start=True,
                    stop=True,
                )

            for a in range(N_ATILES):
                for h in range(2):  # two PSUM halves of 2048 support cols
                    ps = pp.tile([ATILE, 4 * 512], f32, tag="ps")
                    for j in range(4):
                        nc.tensor.matmul(
                            ps[:, ts(j, 512)],
                            ft_r[:, ts(a, ATILE)],
                            st_r[:, ds(2048 * h + 512 * j, 512)],
                            start=True,
                            stop=True,
                        )
                    psv = ps[:].rearrange("p (b x) -> p b x", x=BLK)
                    kind = tmap[(a, h)]
                    if kind == DIRECT:
                        res = resp.tile([ATILE, 16], f32)
                        nc.vector.tensor_reduce(
                            res[:], psv, axis=X, op=mybir.AluOpType.max
                        )
                        nc.sync.dma_start(
                            bmax[ts(a, ATILE), ds(16 * h, 16)], res[:]
                        )
                    elif kind == CAST64:
                        s = castp.tile([ATILE, 16, BLK], f16)
                        nc.scalar.copy(s[:], psv)
                        t = treep.tile([ATILE, 16, 64], f16)
                        nc.vector.tensor_tensor(
                            t[:], s[:, :, 0:64], s[:, :, 64:128],
                            op=mybir.AluOpType.max,
                        )
                        nc.sync.dma_start(bm64[ts(a, ATILE), h, :, :], t[:])
                    else:  # RAW diagonal tile: ship the cast, host reduces
                        s = castp.tile([ATILE, 16, BLK], f16)
                        nc.scalar.copy(s[:], psv)
                        nc.sync.dma_start(braw[ts(a, ATILE), :, :], s[:])

    nc.compile()
    return nc


def _get_program():
    if "nc" not in _CACHE:
        _CACHE["nc"] = _build_program()
    return _CACHE["nc"]


def _make_in_maps(feats, feats_s):
    fs = feats_s.reshape(B * TOPK, C)
    in_maps = []
    for c in range(N_CORES):
        ftc = np.ascontiguousarray(np.roll(feats, -A_ROT * c, axis=0).T).astype(
            np.float16
        )
        stc = np.ascontiguousarray(fs[S_LOC * c : S_LOC * (c + 1)].T).astype(
            np.float16
        )
        in_maps.append({"ft": ftc, "st": stc})
    return in_maps


def run_device(feats, feats_s, trace=False, tmpdir=None):
    """Run the SPMD program; return (blk_smax [B, P_IDS], pos_sim [B], raw)."""
    from concourse.bass_utils import run_bass_kernel_spmd

    nc = _get_program()
    in_maps = _make_in_maps(feats, feats_s)
    kw = {}
    if trace:
        kw = dict(trace=True, tmpdir=tmpdir)
    r = run_bass_kernel_spmd(nc, in_maps, list(range(N_CORES)), **kw)

    tmap = _schedule()
    blk_smax = np.empty((B, P_IDS), np.float64)
    pos_sim = np.empty((B,), np.float64)
    j = np.arange(ATILE)
    for c in range(N_CORES):
        bm = np.array(r.results[c]["bmax"])    # [B, 32]; valid on direct halves
        b64 = np.asarray(r.results[c]["bm64"])  # [B, 2, 16, 64] fp16
        raw = np.asarray(r.results[c]["braw"]).reshape(4, ATILE, 16, BLK)
        for a in range(N_ATILES):
            for h in range(2):
                kind = tmap[(a, h)]
                sl = slice(128 * a, 128 * (a + 1))
                cl = slice(16 * h, 16 * (h + 1))
                if kind == CAST64:
                    bm[sl, cl] = b64[sl, h].astype(np.float32).max(axis=2)
                elif kind == RAW:
                    bm[sl, cl] = raw[a].astype(np.float32).max(axis=2)
        blk_smax[:, NBLK_LOC * c : NBLK_LOC * (c + 1)] = np.roll(
            bm, A_ROT * c, axis=0
        )
        # band min for anchors [512c, 512c+512) from the 4 raw diagonal tiles
        for a in range(4):
            band = raw[a][j, (a % 2) * 8 + j // K_INST, :]   # [128, 128]
            pos_sim[A_ROT * c + ATILE * a + j] = band.astype(np.float32).min(
                axis=1
            )
    return blk_smax, pos_sim, r


def _loss_from_reductions(blk_smax, pos_sim, labels):
    e = np.exp(blk_smax / TEMP)             # [B, P_IDS] block max of exp
    own = e[np.arange(B), labels]
    neg = e.sum(axis=1) - own
    pos = np.exp(pos_sim / TEMP)
    loss = -np.log(pos / (pos + neg + EPS) + EPS)
    return np.float32(loss.mean())


def _numpy_fallback(feats, feats_s, labels):
    # Exact mirror of the reference, host-only. Safety net for label
    # patterns other than arange(B)//K_INST (never produced by setup_inputs).
    fs = feats_s.reshape(B * TOPK, C)
    sim = feats.astype(np.float64) @ fs.astype(np.float64).T
    e = np.exp(sim / TEMP).reshape(B, P_IDS, BLK)
    pos = e[np.arange(B), labels].min(axis=1)
    bm = e.max(axis=2)
    neg = bm.sum(axis=1) - bm[np.arange(B), labels]
    out = -np.log(pos / (pos + neg + EPS) + EPS)
    return np.float32(out.mean())


def kernel(**inputs):
    feats = np.ascontiguousarray(np.asarray(inputs["feats"], dtype=np.float32))
    feats_s = np.ascontiguousarray(np.asarray(inputs["feats_s"], dtype=np.float32))
    labels = np.asarray(inputs["labels"]).astype(np.int64)

    blk_smax, pos_sim, _ = run_device(feats, feats_s)

    if not np.array_equal(labels, np.arange(B, dtype=np.int64) // K_INST):
        return _numpy_fallback(feats, feats_s, labels)
    return _loss_from_reductions(blk_smax, pos_sim, labels)


# revision 4
# speedup vs baseline: 1.2257x; 1.0985x over previous
"""Trainium2 Bass kernel for nn_CriterionLP (hardest-pos/hardest-neg LP loss).

Math (reference):
    sim  = feats @ feats_s.reshape(B*TOPK, C).T          # [B, B*TOPK]
    blk  = exp(sim/T).reshape(B, P_IDS, K_INST*TOPK)
    pos  = min over own identity block                    # exp is monotone =>
    nmax = max over each identity block                   #   reduce raw sim, exp later
    loss = mean(-log(pos / (pos + sum_{j!=pid} nmax_j + eps) + eps))

Device strategy (8 NeuronCores, SPMD — one program, per-core data):
  * Shard the support dim: core c owns support columns [4096c, 4096(c+1));
    anchors are rotated by 512c per core so the program is identical per core.
  * Per core: [C=128 x 4096] @ [C=128 x 4096] fp16 matmuls into PSUM, tiled
    as [128 anchors, 1024 support] quarter-tiles (2 PSUM banks, 4-deep
    rotation so the matmul->drain->matmul round trip never stalls the
    drain engines).
  * Each quarter is drained by one of three balanced paths:
      DIRECT: DVE segmented tensor_reduce(max) straight from PSUM
      CAST64: ACT copies PSUM f32 -> SBUF fp16, one DVE fp16 tensor_tensor
              fold 128->64, ship [128,8,64]; host finishes the 64->1 max
              (fp16 rounding commutes with max).
      RAW:    the 4 diagonal quarters (which also need the own-block band
              min) are ACT-cast and shipped raw; host does max + min.
    A greedy schedule balances DVE vs ACT finishing times.
  * Host: gather, exp at the [B, 256] level, assemble the scalar loss.
"""

import numpy as np

B = 4096
C = 128
TOPK = 8
K_INST = 16
P_IDS = B // K_INST            # 256 identity blocks
BLK = K_INST * TOPK            # 128 support cols per identity block
TEMP = 0.05
EPS = 1e-6

N_CORES = 8
S_LOC = B * TOPK // N_CORES    # 4096 support cols per core
NBLK_LOC = S_LOC // BLK        # 32 identity blocks per core
A_ROT = B // N_CORES           # 512: per-core anchor rotation
ATILE = 128                    # anchors per tile (partition dim)
N_ATILES = B // ATILE          # 32
NQ = 4                         # quarter-tiles of 1024 support cols
QBLK = 8                       # identity blocks per quarter

_CACHE = {}

# Measured/scaled per-quarter engine costs (ns).
DVE_DIRECT = 1302.0   # tensor_reduce [128, 8x128] f32 from PSUM
ACT_CAST = 1120.0     # ACT copy PSUM f32 -> SBUF fp16 [128, 1024]
DVE_FOLD1 = 440.0     # fp16 TT max fold 128 -> 64 from SBUF

DIRECT, CAST64, RAW = 0, 1, 2


def _schedule():
    """Greedy DVE/ACT balance; must match between build and host gather."""
    tmap = {}
    dve, act = 0.0, 0.0
    for a in range(N_ATILES):
        for q in range(NQ):
            if a < 4 and q == a:          # diagonal quarter (own-block band)
                tmap[(a, q)] = RAW
                act += ACT_CAST
                continue
            m_dir = max(dve + DVE_DIRECT, act)
            m_cast = max(dve + DVE_FOLD1, act + ACT_CAST)
            if m_dir <= m_cast:
                tmap[(a, q)] = DIRECT
                dve += DVE_DIRECT
            else:
                tmap[(a, q)] = CAST64
                dve += DVE_FOLD1
                act += ACT_CAST
    return tmap


def _build_program():
    import concourse.tile as tile
    from concourse import bacc, mybir
    from concourse.bass import ds, ts

    nc = bacc.Bacc(
        "TRN2", target_bir_lowering=False, debug=False, num_devices=N_CORES
    )
    f32 = mybir.dt.float32
    f16 = mybir.dt.float16
    X = mybir.AxisListType.X

    ft = nc.dram_tensor("ft", [C, B], f16, kind="ExternalInput").ap()
    st = nc.dram_tensor("st", [C, S_LOC], f16, kind="ExternalInput").ap()
    bmax = nc.dram_tensor("bmax", [B, NBLK_LOC], f32, kind="ExternalOutput").ap()
    bm64 = nc.dram_tensor("bm64", [B, NQ, QBLK, 64], f16, kind="ExternalOutput").ap()
    braw = nc.dram_tensor("braw", [4 * ATILE, QBLK, BLK], f16, kind="ExternalOutput").ap()

    tmap = _schedule()

    with tile.TileContext(nc) as tc:
        with (
            tc.tile_pool(name="inp", bufs=1) as inp,
            tc.tile_pool(name="res", bufs=6) as resp,
            tc.tile_pool(name="cast", bufs=8) as castp,
            tc.tile_pool(name="tree", bufs=6) as treep,
            tc.tile_pool(name="psum", bufs=4, space="PSUM") as pp,
        ):
            ft_r = inp.tile([C, B], f16)
            st_r = inp.tile([C, S_LOC], f16)
            # First anchor tile needs ft[:, 0:128] and st[:, :]; emit the
            # leading pieces first so the pipeline starts as soon as possible.
            nc.sync.dma_start(ft_r[:, ts(0, 512)], ft[:, ts(0, 512)])
            for qd in range(4):
                nc.sync.dma_start(
                    st_r[:, ts(qd, S_LOC // 4)], st[:, ts(qd, S_LOC // 4)]
                )
            for qd in range(1, 8):
                nc.sync.dma_start(ft_r[:, ts(qd, 512)], ft[:, ts(qd, 512)])

            # PE HAM warm-up during the input-DMA window (zeroed on the idle
            # GpSimd engine so ACT's preamble doesn't gate it).
            warm = inp.tile([C, 512], f16)
            nc.gpsimd.memset(warm[:], 0.0)
            ps_w = pp.tile([ATILE, 1024], f32, tag="ps")
            for i in range(5):
                nc.tensor.matmul(
                    ps_w[:, ts(i % 2, 512)],
                    warm[:, 0:ATILE],
                    warm[:],
                    start=True,
                    stop=True,
                )

            for a in range(N_ATILES):
                for q in range(NQ):
                    ps = pp.tile([ATILE, 1024], f32, tag="ps")
                    for j in range(2):
                        nc.tensor.matmul(
                            ps[:, ts(j, 512)],
                            ft_r[:, ts(a, ATILE)],
                            st_r[:, ds(1024 * q + 512 * j, 512)],
                            start=True,
                            stop=True,
                        )
                    psv = ps[:].rearrange("p (b x) -> p b x", x=BLK)
                    kind = tmap[(a, q)]
                    if kind == DIRECT:
                        res = resp.tile([ATILE, QBLK], f32)
                        nc.vector.tensor_reduce(
                            res[:], psv, axis=X, op=mybir.AluOpType.max
                        )
                        nc.sync.dma_start(
                            bmax[ts(a, ATILE), ds(QBLK * q, QBLK)], res[:]
                        )
                    elif kind == CAST64:
                        s = castp.tile([ATILE, QBLK, BLK], f16)
                        nc.scalar.copy(s[:], psv)
                        t = treep.tile([ATILE, QBLK, 64], f16)
                        nc.vector.tensor_tensor(
                            t[:], s[:, :, 0:64], s[:, :, 64:128],
                            op=mybir.AluOpType.max,
                        )
                        nc.sync.dma_start(bm64[ts(a, ATILE), q, :, :], t[:])
                    else:  # RAW diagonal quarter: ship the cast, host reduces
                        s = castp.tile([ATILE, QBLK, BLK], f16)
                        nc.scalar.copy(s[:], psv)
                        nc.sync.dma_start(braw[ts(a, ATILE), :, :], s[:])

    nc.compile()
    return nc


def _get_program():
    if "nc" not in _CACHE:
        _CACHE["nc"] = _build_program()
    return _CACHE["nc"]


def _make_in_maps(feats, feats_s):
    fs = feats_s.reshape(B * TOPK, C)
    in_maps = []
    for c in range(N_CORES):
        ftc = np.ascontiguousarray(np.roll(feats, -A_ROT * c, axis=0).T).astype(
            np.float16
        )
        stc = np.ascontiguousarray(fs[S_LOC * c : S_LOC * (c + 1)].T).astype(
            np.float16
        )
        in_maps.append({"ft": ftc, "st": stc})
    return in_maps


def run_device(feats, feats_s, trace=False, tmpdir=None):
    """Run the SPMD program; return (blk_smax [B, P_IDS], pos_sim [B], raw)."""
    from concourse.bass_utils import run_bass_kernel_spmd

    nc = _get_program()
    in_maps = _make_in_maps(feats, feats_s)
    kw = {}
    if trace:
        kw = dict(trace=True, tmpdir=tmpdir)
    r = run_bass_kernel_spmd(nc, in_maps, list(range(N_CORES)), **kw)

    tmap = _schedule()
    blk_smax = np.empty((B, P_IDS), np.float64)
    pos_sim = np.empty((B,), np.float64)
    j = np.arange(ATILE)
    for c in range(N_CORES):
        bm = np.array(r.results[c]["bmax"])    # [B, 32]; valid on direct qtrs
        b64 = np.asarray(r.results[c]["bm64"])  # [B, 4, 8, 64] fp16
        raw = np.asarray(r.results[c]["braw"]).reshape(4, ATILE, QBLK, BLK)
        for a in range(N_ATILES):
            for q in range(NQ):
                kind = tmap[(a, q)]
                if kind == DIRECT:
                    continue
                sl = slice(128 * a, 128 * (a + 1))
                cl = slice(QBLK * q, QBLK * (q + 1))
                if kind == CAST64:
                    bm[sl, cl] = b64[sl, q].astype(np.float32).max(axis=2)
                else:
                    bm[sl, cl] = raw[a].astype(np.float32).max(axis=2)
        blk_smax[:, NBLK_LOC * c : NBLK_LOC * (c + 1)] = np.roll(
            bm, A_ROT * c, axis=0
        )
        # band min for anchors [512c, 512c+512) from the 4 raw diag quarters
        for a in range(4):
            band = raw[a][j, j // K_INST, :]   # [128, 128] own-block rows
            pos_sim[A_ROT * c + ATILE * a + j] = band.astype(np.float32).min(
                axis=1
            )
    return blk_smax, pos_sim, r


def _loss_from_reductions(blk_smax, pos_sim, labels):
    e = np.exp(blk_smax / TEMP)             # [B, P_IDS] block max of exp
    own = e[np.arange(B), labels]
    neg = e.sum(axis=1) - own
    pos = np.exp(pos_sim / TEMP)
    loss = -np.log(pos / (pos + neg + EPS) + EPS)
    return np.float32(loss.mean())


def _numpy_fallback(feats, feats_s, labels):
    # Exact mirror of the reference, host-only. Safety net for label
    # patterns other than arange(B)//K_INST (never produced by setup_inputs).
    fs = feats_s.reshape(B * TOPK, C)
    sim = feats.astype(np.float64) @ fs.astype(np.float64).T
    e = np.exp(sim / TEMP).reshape(B, P_IDS, BLK)
    pos = e[np.arange(B), labels].min(axis=1)
    bm = e.max(axis=2)
    neg = bm.sum(axis=1) - bm[np.arange(B), labels]
    out = -np.log(pos / (pos + neg + EPS) + EPS)
    return np.float32(out.mean())


def kernel(**inputs):
    feats = np.ascontiguousarray(np.asarray(inputs["feats"], dtype=np.float32))
    feats_s = np.ascontiguousarray(np.asarray(inputs["feats_s"], dtype=np.float32))
    labels = np.asarray(inputs["labels"]).astype(np.int64)

    blk_smax, pos_sim, _ = run_device(feats, feats_s)

    if not np.array_equal(labels, np.arange(B, dtype=np.int64) // K_INST):
        return _numpy_fallback(feats, feats_s, labels)
    return _loss_from_reductions(blk_smax, pos_sim, labels)


# revision 6
# speedup vs baseline: 1.2371x; 1.0093x over previous
"""Trainium2 Bass kernel for nn_CriterionLP (hardest-pos/hardest-neg LP loss).

Math (reference):
    sim  = feats @ feats_s.reshape(B*TOPK, C).T          # [B, B*TOPK]
    blk  = exp(sim/T).reshape(B, P_IDS, K_INST*TOPK)
    pos  = min over own identity block                    # exp is monotone =>
    nmax = max over each identity block                   #   reduce raw sim, exp later
    loss = mean(-log(pos / (pos + sum_{j!=pid} nmax_j + eps) + eps))

Device strategy (8 NeuronCores, SPMD — one program, per-core data):
  * Shard the support dim: core c owns support columns [4096c, 4096(c+1));
    anchors are rotated by 512c per core so the program is identical per core.
  * Per core: [C=128 x 4096] @ [C=128 x 4096] fp16 matmuls into PSUM, tiled
    as [128 anchors, 1024 support] quarter-tiles (2 PSUM banks, 4-deep
    rotation so the matmul->drain->matmul round trip never stalls the
    drain engines).
  * Each quarter is drained by one of three balanced paths:
      DIRECT: DVE segmented tensor_reduce(max) straight from PSUM
      CAST64: ACT copies PSUM f32 -> SBUF fp16, one DVE fp16 tensor_tensor
              fold 128->64, ship [128,8,64]; host finishes the 64->1 max
              (fp16 rounding commutes with max).
      RAW:    the 4 diagonal quarters (which also need the own-block band
              min) are ACT-cast and shipped raw; host does max + min.
    A greedy schedule balances DVE vs ACT finishing times.
  * Host: gather, exp at the [B, 256] level, assemble the scalar loss.
"""

import numpy as np

B = 4096
C = 128
TOPK = 8
K_INST = 16
P_IDS = B // K_INST            # 256 identity blocks
BLK = K_INST * TOPK            # 128 support cols per identity block
TEMP = 0.05
EPS = 1e-6

N_CORES = 8
S_LOC = B * TOPK // N_CORES    # 4096 support cols per core
NBLK_LOC = S_LOC // BLK        # 32 identity blocks per core
A_ROT = B // N_CORES           # 512: per-core anchor rotation
ATILE = 128                    # anchors per tile (partition dim)
N_ATILES = B // ATILE          # 32
NQ = 4                         # quarter-tiles of 1024 support cols
QBLK = 8                       # identity blocks per quarter

_CACHE = {}

# Measured per-quarter engine costs (ns), from the v4 NTFF trace.
DVE_DIRECT = 1218.0   # tensor_reduce [128, 8x128] f32 from PSUM
ACT_CAST = 1156.0     # ACT copy PSUM f32 -> SBUF fp16 [128, 1024]
DVE_FOLD1 = 418.0     # fp16 TT max fold 128 -> 64 from SBUF

DIRECT, CAST64, RAW = 0, 1, 2


def _schedule():
    """Greedy DVE/ACT balance; must match between build and host gather."""
    tmap = {}
    dve, act = 0.0, 0.0
    for a in range(N_ATILES):
        for q in range(NQ):
            if a < 4 and q == a:          # diagonal quarter (own-block band)
                tmap[(a, q)] = RAW
                act += ACT_CAST
                continue
            m_dir = max(dve + DVE_DIRECT, act)
            m_cast = max(dve + DVE_FOLD1, act + ACT_CAST)
            if m_dir <= m_cast:
                tmap[(a, q)] = DIRECT
                dve += DVE_DIRECT
            else:
                tmap[(a, q)] = CAST64
                dve += DVE_FOLD1
                act += ACT_CAST
    return tmap


def _build_program():
    import concourse.tile as tile
    from concourse import bacc, mybir
    from concourse.bass import ds, ts

    nc = bacc.Bacc(
        "TRN2", target_bir_lowering=False, debug=False, num_devices=N_CORES
    )
    f32 = mybir.dt.float32
    f16 = mybir.dt.float16
    X = mybir.AxisListType.X

    ft = nc.dram_tensor("ft", [C, B], f16, kind="ExternalInput").ap()
    st = nc.dram_tensor("st", [C, S_LOC], f16, kind="ExternalInput").ap()
    bmax = nc.dram_tensor("bmax", [B, NBLK_LOC], f32, kind="ExternalOutput").ap()
    bm64 = nc.dram_tensor("bm64", [B, NQ, QBLK, 64], f16, kind="ExternalOutput").ap()
    braw = nc.dram_tensor("braw", [4 * ATILE, QBLK, BLK], f16, kind="ExternalOutput").ap()

    tmap = _schedule()

    with tile.TileContext(nc) as tc:
        with (
            tc.tile_pool(name="inp", bufs=1) as inp,
            tc.tile_pool(name="res", bufs=6) as resp,
            tc.tile_pool(name="cast", bufs=8) as castp,
            tc.tile_pool(name="tree", bufs=6) as treep,
            tc.tile_pool(name="psum", bufs=4, space="PSUM") as pp,
        ):
            ft_r = inp.tile([C, B], f16)
            st_r = inp.tile([C, S_LOC], f16)
            # First anchor tile needs ft[:, 0:128] and st[:, :]; emit the
            # leading pieces first so the pipeline starts as soon as possible.
            nc.sync.dma_start(ft_r[:, ts(0, 512)], ft[:, ts(0, 512)])
            for qd in range(4):
                nc.sync.dma_start(
                    st_r[:, ts(qd, S_LOC // 4)], st[:, ts(qd, S_LOC // 4)]
                )
            for qd in range(1, 8):
                nc.sync.dma_start(ft_r[:, ts(qd, 512)], ft[:, ts(qd, 512)])

            # PE HAM warm-up during the input-DMA window (zeroed on the idle
            # GpSimd engine so ACT's preamble doesn't gate it).
            warm = inp.tile([C, 512], f16)
            nc.gpsimd.memset(warm[:], 0.0)
            ps_w = pp.tile([ATILE, 1024], f32, tag="ps")
            for i in range(3):
                nc.tensor.matmul(
                    ps_w[:, ts(i % 2, 512)],
                    warm[:, 0:ATILE],
                    warm[:],
                    start=True,
                    stop=True,
                )

            for a in range(N_ATILES):
                for q in range(NQ):
                    ps = pp.tile([ATILE, 1024], f32, tag="ps")
                    for j in range(2):
                        nc.tensor.matmul(
                            ps[:, ts(j, 512)],
                            ft_r[:, ts(a, ATILE)],
                            st_r[:, ds(1024 * q + 512 * j, 512)],
                            start=True,
                            stop=True,
                        )
                    psv = ps[:].rearrange("p (b x) -> p b x", x=BLK)
                    kind = tmap[(a, q)]
                    if kind == DIRECT:
                        res = resp.tile([ATILE, QBLK], f32)
                        nc.vector.tensor_reduce(
                            res[:], psv, axis=X, op=mybir.AluOpType.max
                        )
                        nc.sync.dma_start(
                            bmax[ts(a, ATILE), ds(QBLK * q, QBLK)], res[:]
                        )
                    elif kind == CAST64:
                        s = castp.tile([ATILE, QBLK, BLK], f16)
                        nc.scalar.copy(s[:], psv)
                        t = treep.tile([ATILE, QBLK, 64], f16)
                        nc.vector.tensor_tensor(
                            t[:], s[:, :, 0:64], s[:, :, 64:128],
                            op=mybir.AluOpType.max,
                        )
                        nc.sync.dma_start(bm64[ts(a, ATILE), q, :, :], t[:])
                    else:  # RAW diagonal quarter: ship the cast, host reduces
                        s = castp.tile([ATILE, QBLK, BLK], f16)
                        nc.scalar.copy(s[:], psv)
                        nc.sync.dma_start(braw[ts(a, ATILE), :, :], s[:])

    nc.compile()
    return nc


def _get_program():
    if "nc" not in _CACHE:
        _CACHE["nc"] = _build_program()
    return _CACHE["nc"]


def _make_in_maps(feats, feats_s):
    fs = feats_s.reshape(B * TOPK, C)
    in_maps = []
    for c in range(N_CORES):
        ftc = np.ascontiguousarray(np.roll(feats, -A_ROT * c, axis=0).T).astype(
            np.float16
        )
        stc = np.ascontiguousarray(fs[S_LOC * c : S_LOC * (c + 1)].T).astype(
            np.float16
        )
        in_maps.append({"ft": ftc, "st": stc})
    return in_maps


def run_device(feats, feats_s, trace=False, tmpdir=None):
    """Run the SPMD program; return (blk_smax [B, P_IDS], pos_sim [B], raw)."""
    from concourse.bass_utils import run_bass_kernel_spmd

    nc = _get_program()
    in_maps = _make_in_maps(feats, feats_s)
    kw = {}
    if trace:
        kw = dict(trace=True, tmpdir=tmpdir)
    r = run_bass_kernel_spmd(nc, in_maps, list(range(N_CORES)), **kw)

    tmap = _schedule()
    blk_smax = np.empty((B, P_IDS), np.float64)
    pos_sim = np.empty((B,), np.float64)
    j = np.arange(ATILE)
    for c in range(N_CORES):
        bm = np.array(r.results[c]["bmax"])    # [B, 32]; valid on direct qtrs
        b64 = np.asarray(r.results[c]["bm64"])  # [B, 4, 8, 64] fp16
        raw = np.asarray(r.results[c]["braw"]).reshape(4, ATILE, QBLK, BLK)
        for a in range(N_ATILES):
            for q in range(NQ):
                kind = tmap[(a, q)]
                if kind == DIRECT:
                    continue
                sl = slice(128 * a, 128 * (a + 1))
                cl = slice(QBLK * q, QBLK * (q + 1))
                if kind == CAST64:
                    bm[sl, cl] = b64[sl, q].astype(np.float32).max(axis=2)
                else:
                    bm[sl, cl] = raw[a].astype(np.float32).max(axis=2)
        blk_smax[:, NBLK_LOC * c : NBLK_LOC * (c + 1)] = np.roll(
            bm, A_ROT * c, axis=0
        )
        # band min for anchors [512c, 512c+512) from the 4 raw diag quarters
        for a in range(4):
            band = raw[a][j, j // K_INST, :]   # [128, 128] own-block rows
            pos_sim[A_ROT * c + ATILE * a + j] = band.astype(np.float32).min(
                axis=1
            )
    return blk_smax, pos_sim, r


def _loss_from_reductions(blk_smax, pos_sim, labels):
    e = np.exp(blk_smax / TEMP)             # [B, P_IDS] block max of exp
    own = e[np.arange(B), labels]
    neg = e.sum(axis=1) - own
    pos = np.exp(pos_sim / TEMP)
    loss = -np.log(pos / (pos + neg + EPS) + EPS)
    return np.float32(loss.mean())


def _numpy_fallback(feats, feats_s, labels):
    # Exact mirror of the reference, host-only. Safety net for label
    # patterns other than arange(B)//K_INST (never produced by setup_inputs).
    fs = feats_s.reshape(B * TOPK, C)
    sim = feats.astype(np.float64) @ fs.astype(np.float64).T
    e = np.exp(sim / TEMP).reshape(B, P_IDS, BLK)
    pos = e[np.arange(B), labels].min(axis=1)
    bm = e.max(axis=2)
    neg = bm.sum(axis=1) - bm[np.arange(B), labels]
    out = -np.log(pos / (pos + neg + EPS) + EPS)
    return np.float32(out.mean())


def kernel(**inputs):
    feats = np.ascontiguousarray(np.asarray(inputs["feats"], dtype=np.float32))
    feats_s = np.ascontiguousarray(np.asarray(inputs["feats_s"], dtype=np.float32))
    labels = np.asarray(inputs["labels"]).astype(np.int64)

    blk_smax, pos_sim, _ = run_device(feats, feats_s)

    if not np.array_equal(labels, np.arange(B, dtype=np.int64) // K_INST):
        return _numpy_fallback(feats, feats_s, labels)
    return _loss_from_reductions(blk_smax, pos_sim, labels)


# revision 10
# speedup vs baseline: 1.4189x; 1.1469x over previous
"""Trainium2 Bass kernel for nn_CriterionLP (hardest-pos/hardest-neg LP loss).

Math (reference):
    sim  = feats @ feats_s.reshape(B*TOPK, C).T          # [B, B*TOPK]
    blk  = exp(sim/T).reshape(B, P_IDS, K_INST*TOPK)
    pos  = min over own identity block                    # exp is monotone =>
    nmax = max over each identity block                   #   reduce raw sim, exp later
    loss = mean(-log(pos / (pos + sum_{j!=pid} nmax_j + eps) + eps))

Device strategy (8 NeuronCores, SPMD — one program, per-core data):
  * Shard the support dim: core c owns support columns [4096c, 4096(c+1));
    anchors are rotated by 512c per core so the program is identical per core.
  * Per core: [C=128 x 4096] @ [C=128 x 4096] fp16 matmuls into PSUM, tiled
    as [128 anchors, 1024 support] quarter-tiles (2 PSUM banks, 4-deep
    rotation so the matmul->drain->matmul round trip never stalls the
    drain engines).
  * Each quarter is drained by one of three balanced paths:
      DIRECT: DVE segmented tensor_reduce(max) straight from PSUM
      CAST64: ACT copies PSUM f32 -> SBUF fp16, one DVE fp16 tensor_tensor
              fold 128->64, ship [128,8,64]; host finishes the 64->1 max
              (fp16 rounding commutes with max).
      RAW:    the 4 diagonal quarters (which also need the own-block band
              min) are ACT-cast and shipped raw; host does max + min.
    A greedy schedule balances DVE vs ACT finishing times.
  * Host: gather, exp at the [B, 256] level, assemble the scalar loss.
"""

import numpy as np

B = 4096
C = 128
TOPK = 8
K_INST = 16
P_IDS = B // K_INST            # 256 identity blocks
BLK = K_INST * TOPK            # 128 support cols per identity block
TEMP = 0.05
EPS = 1e-6

N_CORES = 8
S_LOC = B * TOPK // N_CORES    # 4096 support cols per core
NBLK_LOC = S_LOC // BLK        # 32 identity blocks per core
A_ROT = B // N_CORES           # 512: per-core anchor rotation
ATILE = 128                    # anchors per tile (partition dim)
N_ATILES = B // ATILE          # 32
NQ = 4                         # quarter-tiles of 1024 support cols
QBLK = 8                       # identity blocks per quarter

_CACHE = {}

# Measured per-quarter engine costs (ns), from the v4 NTFF trace.
DVE_DIRECT = 1218.0   # tensor_reduce [128, 8x128] f32 from PSUM
ACT_CAST = 1156.0     # ACT copy PSUM f32 -> SBUF fp16 [128, 1024]
DVE_FOLD1 = 418.0     # fp16 TT max fold 128 -> 64 from SBUF

DIRECT, CAST64, RAW = 0, 1, 2
MAX_RAW = 44          # raw-shipped quarters (incl. the 4 diagonal ones)


def _schedule():
    """Greedy DVE/ACT balance; must match between build and host gather.
    Returns (tmap, raw_slots): tile kinds and each RAW quarter's slot in
    the braw output tensor."""
    tmap = {}
    raw_slots = {}
    dve, act = 0.0, 0.0
    cast_idx = 0
    for a in range(N_ATILES):
        for q in range(NQ):
            if a < 4 and q == a:          # diagonal quarter (own-block band)
                kind = RAW
                act += ACT_CAST
            else:
                # 3 of every 5 cast-path quarters ship raw (no DVE fold),
                # bounded by the DMA/host budget.
                will_raw = cast_idx % 5 < 3 and len(raw_slots) < MAX_RAW
                fold = 0.0 if will_raw else DVE_FOLD1
                m_dir = max(dve + DVE_DIRECT, act)
                m_cast = max(dve + fold, act + ACT_CAST)
                if m_dir <= m_cast:
                    kind = DIRECT
                    dve += DVE_DIRECT
                else:
                    kind = RAW if will_raw else CAST64
                    dve += fold
                    act += ACT_CAST
                    cast_idx += 1
            if kind == RAW:
                raw_slots[(a, q)] = len(raw_slots)
            tmap[(a, q)] = kind
    return tmap, raw_slots


def _build_program():
    import concourse.tile as tile
    from concourse import bacc, mybir
    from concourse.bass import ds, ts

    nc = bacc.Bacc(
        "TRN2", target_bir_lowering=False, debug=False, num_devices=N_CORES
    )
    f32 = mybir.dt.float32
    f16 = mybir.dt.float16
    X = mybir.AxisListType.X

    ft = nc.dram_tensor("ft", [C, B], f16, kind="ExternalInput").ap()
    st = nc.dram_tensor("st", [C, S_LOC], f16, kind="ExternalInput").ap()
    tmap, raw_slots = _schedule()
    nraw = len(raw_slots)

    bmax = nc.dram_tensor("bmax", [B, NBLK_LOC], f32, kind="ExternalOutput").ap()
    bm64 = nc.dram_tensor("bm64", [B, NQ, QBLK, 64], f16, kind="ExternalOutput").ap()
    braw = nc.dram_tensor("braw", [nraw * ATILE, QBLK, BLK], f16, kind="ExternalOutput").ap()

    with tile.TileContext(nc) as tc:
        with (
            tc.tile_pool(name="inp", bufs=1) as inp,
            tc.tile_pool(name="res", bufs=6) as resp,
            tc.tile_pool(name="cast", bufs=8) as castp,
            tc.tile_pool(name="tree", bufs=6) as treep,
            tc.tile_pool(name="psum", bufs=4, space="PSUM") as pp,
        ):
            ft_r = inp.tile([C, B], f16)
            st_r = inp.tile([C, S_LOC], f16)
            # First anchor tile needs ft[:, 0:128] and st[:, :]; emit the
            # leading pieces first so the pipeline starts as soon as possible.
            nc.sync.dma_start(ft_r[:, ts(0, 512)], ft[:, ts(0, 512)])
            for qd in range(4):
                nc.sync.dma_start(
                    st_r[:, ts(qd, S_LOC // 4)], st[:, ts(qd, S_LOC // 4)]
                )
            for qd in range(1, 8):
                nc.sync.dma_start(ft_r[:, ts(qd, 512)], ft[:, ts(qd, 512)])

            # PE HAM warm-up during the input-DMA window (zeroed on the idle
            # GpSimd engine so ACT's preamble doesn't gate it).
            warm = inp.tile([C, 512], f16)
            nc.gpsimd.memset(warm[:], 0.0)
            ps_w = pp.tile([ATILE, 1024], f32, tag="ps")
            for i in range(3):
                nc.tensor.matmul(
                    ps_w[:, ts(i % 2, 512)],
                    warm[:, 0:ATILE],
                    warm[:],
                    start=True,
                    stop=True,
                )

            for a in range(N_ATILES):
                for q in range(NQ):
                    ps = pp.tile([ATILE, 1024], f32, tag="ps")
                    for j in range(2):
                        nc.tensor.matmul(
                            ps[:, ts(j, 512)],
                            ft_r[:, ts(a, ATILE)],
                            st_r[:, ds(1024 * q + 512 * j, 512)],
                            start=True,
                            stop=True,
                        )
                    psv = ps[:].rearrange("p (b x) -> p b x", x=BLK)
                    kind = tmap[(a, q)]
                    if kind == DIRECT:
                        res = resp.tile([ATILE, QBLK], f32)
                        nc.vector.tensor_reduce(
                            res[:], psv, axis=X, op=mybir.AluOpType.max
                        )
                        nc.sync.dma_start(
                            bmax[ts(a, ATILE), ds(QBLK * q, QBLK)], res[:]
                        )
                    elif kind == CAST64:
                        s = castp.tile([ATILE, QBLK, BLK], f16)
                        nc.scalar.copy(s[:], psv)
                        t = treep.tile([ATILE, QBLK, 64], f16)
                        nc.vector.tensor_tensor(
                            t[:], s[:, :, 0:64], s[:, :, 64:128],
                            op=mybir.AluOpType.max,
                        )
                        nc.sync.dma_start(bm64[ts(a, ATILE), q, :, :], t[:])
                    else:  # RAW quarter: ship the cast, host reduces.
                        # Alternate the big transfers between the HWDGE sync
                        # queue and the idle GpSimd SWDGE queue.
                        slot = raw_slots[(a, q)]
                        s = castp.tile([ATILE, QBLK, BLK], f16)
                        nc.scalar.copy(s[:], psv)
                        eng = nc.gpsimd if slot % 2 else nc.sync
                        eng.dma_start(braw[ts(slot, ATILE), :, :], s[:])

    nc.compile()
    return nc


def _get_program():
    if "nc" not in _CACHE:
        _CACHE["nc"] = _build_program()
    return _CACHE["nc"]


def _make_in_maps(feats, feats_s):
    fs = feats_s.reshape(B * TOPK, C)
    in_maps = []
    for c in range(N_CORES):
        ftc = np.ascontiguousarray(np.roll(feats, -A_ROT * c, axis=0).T).astype(
            np.float16
        )
        stc = np.ascontiguousarray(fs[S_LOC * c : S_LOC * (c + 1)].T).astype(
            np.float16
        )
        in_maps.append({"ft": ftc, "st": stc})
    return in_maps


def run_device(feats, feats_s, trace=False, tmpdir=None):
    """Run the SPMD program; return (blk_smax [B, P_IDS], pos_sim [B], raw)."""
    from concourse.bass_utils import run_bass_kernel_spmd

    nc = _get_program()
    in_maps = _make_in_maps(feats, feats_s)
    kw = {}
    if trace:
        kw = dict(trace=True, tmpdir=tmpdir)
    r = run_bass_kernel_spmd(nc, in_maps, list(range(N_CORES)), **kw)

    tmap, raw_slots = _schedule()
    nraw = len(raw_slots)
    blk_smax = np.empty((B, P_IDS), np.float64)
    pos_sim = np.empty((B,), np.float64)
    j = np.arange(ATILE)
    for c in range(N_CORES):
        bm = np.array(r.results[c]["bmax"])    # [B, 32]; valid on direct qtrs
        b64 = np.asarray(r.results[c]["bm64"])  # [B, 4, 8, 64] fp16
        raw = np.asarray(r.results[c]["braw"]).reshape(nraw, ATILE, QBLK, BLK)
        for a in range(N_ATILES):
            for q in range(NQ):
                kind = tmap[(a, q)]
                if kind == DIRECT:
                    continue
                sl = slice(128 * a, 128 * (a + 1))
                cl = slice(QBLK * q, QBLK * (q + 1))
                if kind == CAST64:
                    bm[sl, cl] = b64[sl, q].astype(np.float32).max(axis=2)
                else:
                    slot = raw_slots[(a, q)]
                    bm[sl, cl] = raw[slot].astype(np.float32).max(axis=2)
        blk_smax[:, NBLK_LOC * c : NBLK_LOC * (c + 1)] = np.roll(
            bm, A_ROT * c, axis=0
        )
        # band min for anchors [512c, 512c+512) from the 4 raw diag quarters
        for a in range(4):
            slot = raw_slots[(a, a)]
            band = raw[slot][j, j // K_INST, :]   # [128, 128] own-block rows
            pos_sim[A_ROT * c + ATILE * a + j] = band.astype(np.float32).min(
                axis=1
            )
    return blk_smax, pos_sim, r


def _loss_from_reductions(blk_smax, pos_sim, labels):
    e = np.exp(blk_smax / TEMP)             # [B, P_IDS] block max of exp
    own = e[np.arange(B), labels]
    neg = e.sum(axis=1) - own
    pos = np.exp(pos_sim / TEMP)
    loss = -np.log(pos / (pos + neg + EPS) + EPS)
    return np.float32(loss.mean())


def _numpy_fallback(feats, feats_s, labels):
    # Exact mirror of the reference, host-only. Safety net for label
    # patterns other than arange(B)//K_INST (never produced by setup_inputs).
    fs = feats_s.reshape(B * TOPK, C)
    sim = feats.astype(np.float64) @ fs.astype(np.float64).T
    e = np.exp(sim / TEMP).reshape(B, P_IDS, BLK)
    pos = e[np.arange(B), labels].min(axis=1)
    bm = e.max(axis=2)
    neg = bm.sum(axis=1) - bm[np.arange(B), labels]
    out = -np.log(pos / (pos + neg + EPS) + EPS)
    return np.float32(out.mean())


def kernel(**inputs):
    feats = np.ascontiguousarray(np.asarray(inputs["feats"], dtype=np.float32))
    feats_s = np.ascontiguousarray(np.asarray(inputs["feats_s"], dtype=np.float32))
    labels = np.asarray(inputs["labels"]).astype(np.int64)

    blk_smax, pos_sim, _ = run_device(feats, feats_s)

    if not np.array_equal(labels, np.arange(B, dtype=np.int64) // K_INST):
        return _numpy_fallback(feats, feats_s, labels)
    return _loss_from_reductions(blk_smax, pos_sim, labels)
